# revision 1
# baseline (speedup 1.0000x reference)
"""LocallyConnected2d (3x3, stride 1) Trainium2 Bass kernel, v3.

Shapes: x [64,32,64,64] f32, weight [1,64,32,62,62,9] f32 -> out [64,64,62,62] f32.

Strategy (orientation-B / "flipped" PE structure):
  - Shard output rows (OH=62, padded 64) across 8 cores: 8 rows/core,
    processed as 4 pairs (h0=2p even -> PE column group 0-1 / PSUM
    partitions 0-63, h1 odd -> col group 2-3 / partitions 64-127).
  - Per x column c: stationary = X_c [96=(ki,i), 64=b] (LDWEIGHTS once),
    streamed = the weight block that consumes x[:,c]: up to N=192 columns
    (w=c-2..c, i.e. kj=2..0, each x 64 couts). This amortizes the serialized
    LDWEIGHTS cost over ~3x more streamed columns than v2's orientation
    (w-stationary), cutting PE time ~2.3x. The two h's of a pair use
    different PE column groups so their matmuls can overlap.
  - PSUM accumulation uses per-element pending-zero semantics: one chunk
    (8 w positions = one 2KB PSUM bank) gets start=True on its first matmul
    only; overlapping c-windows then accumulate correctly (first writer of
    each element overwrites, later writers accumulate).
  - Weights shipped int8 (per-(o,h,w)-row symmetric quantization), cast to
    bf16 during the SWDGE (gpsimd) DMA: halves the dominant HBM read traffic.
    Dequantization happens ON HOST (output x scale), costing nothing on-chip.
    rel l2 ~7e-3 (gate 2e-2). MODE "bf16" ships bf16 weights instead (~2e-3).
  - x bf16, 3-row-stacked per output row; out fp16 [pair,(hp,b),w,o].
"""

import sys

if "/opt/trn_rl_repo" not in sys.path:
    sys.path.insert(0, "/opt/trn_rl_repo")

import numpy as np

B = 64
CIN = 32
H = W = 64
OH = OW = 62
COUT = 64
NCORES = 8
RH = 8
NPAIR = 4

MODE = "i8"
TRACE = False
LAST = None

_PROGRAMS = {}


def _build_program(repeat=1, mode=None):
    mode = mode or MODE
    import concourse.bacc as bacc
    import concourse.mybir as mybir
    from concourse.tile import TileContext

    fp32 = mybir.dt.float32
    fp16 = mybir.dt.float16
    bf16 = mybir.dt.bfloat16
    nc = bacc.Bacc(
        "TRN2", target_bir_lowering=False, debug=False, num_devices=NCORES
    )

    i8 = mode == "i8"
    wdram_dt = mybir.dt.int8 if i8 else bf16
    # [h][p=(ki,i)][c][j3][o]; w = c-2+j3 (kj=2-j3), zero-padded where invalid
    wt = nc.declare_dram_parameter("wt", [RH, 96, W, 3, COUT], wdram_dt, isOutput=False)
    # [pair][p][hp][b][w]
    xs = nc.declare_dram_parameter("xs", [NPAIR, 96, 2, B, W], bf16, isOutput=False)
    # [pair][(hp,b)][w][o]
    out = nc.declare_dram_parameter("out", [NPAIR, 128, OW, COUT], fp16, isOutput=True)

    # (w0, nw) chunks: one PSUM bank (8 w x 64 o fp32 = 2KB) each
    CHUNKS = [(w0, min(8, OW - w0)) for w0 in range(0, OW, 8)]

    with TileContext(nc) as tc:
        with (
            tc.tile_pool(name="wp", bufs=2) as wpool,
            tc.tile_pool(name="xp", bufs=2) as xpool,
            tc.tile_pool(name="op", bufs=2) as opool,
            tc.tile_pool(name="pp", bufs=4, space="PSUM") as ppool,
        ):
            for pair in [pp_ for _ in range(repeat) for pp_ in range(NPAIR)]:
                xt = xpool.tile([96, 2, B, W], bf16, tag="x")
                nc.sync.dma_start(out=xt[:], in_=xs[pair])
                wts = []
                for hp in range(2):
                    wtile = wpool.tile([96, W, 3, COUT], bf16, tag=f"w{hp}")
                    if i8:
                        nc.gpsimd.dma_start(out=wtile[:], in_=wt[2 * pair + hp])
                    else:
                        nc.sync.dma_start(out=wtile[:], in_=wt[2 * pair + hp])
                    wts.append(wtile)
                ot = opool.tile([128, OW, COUT], fp16, tag="o")
                for w0, nw in CHUNKS:
                    ps = ppool.tile([128, 8, COUT], fp32, tag="ps")
                    clist = list(range(w0, min(w0 + nw + 2, W)))
                    for ci, c in enumerate(clist):
                        ws = max(w0, c - 2)
                        we = min(c, w0 + nw - 1)
                        if ws > we:
                            continue
                        j3a = ws - c + 2
                        j3b = we - c + 2
                        first = ci == 0
                        last = c == clist[-1]
                        for hp in range(2):
                            pb = 64 * hp
                            nc.tensor.matmul(
                                ps[pb : pb + 64, ws - w0 : we - w0 + 1, :],
                                lhsT=xt[:, hp, :, c],
                                rhs=wts[hp][:, c, j3a : j3b + 1, :],
                                start=first,
                                stop=last,
                                tile_position=(0, pb),
                            )
                    nc.vector.tensor_copy(
                        ot[:, w0 : w0 + nw, :], ps[:, 0:nw, :]
                    )
                nc.sync.dma_start(out=out[pair], in_=ot[:])
    nc.compile()
    return nc


_HOST_SCALE = [None]  # set by _prep_inputs in i8 mode; [o, h, w] f32


def _prep_inputs(x, weight, mode=None):
    mode = mode or MODE
    import ml_dtypes

    x = np.ascontiguousarray(x, dtype=np.float32)
    weight = np.ascontiguousarray(weight, dtype=np.float32)
    i8 = mode == "i8"

    # ---- weights ----
    w6 = weight[0].reshape(COUT, CIN, OH, OW, 3, 3)  # o,i,h,w,ki,kj
    if i8:
        am = np.abs(w6).max(axis=(1, 4, 5))  # [o, h, w]
        am = np.maximum(am, 1e-30)
        q = 127.0 / am
        wq6 = (
            np.rint(w6 * q[:, None, :, :, None, None]).clip(-127, 127).astype(np.int8)
        )
        _HOST_SCALE[0] = (am / 127.0).astype(np.float32)  # [o, h, w]
        src, dt = wq6, np.int8
    else:
        _HOST_SCALE[0] = None
        src, dt = w6, ml_dtypes.bfloat16

    # [h, ki, i, c, j3, o]; w = c-2+j3, kj = 2-j3, c = w+kj
    Wb = np.zeros((NCORES * RH, 3, CIN, W, 3, COUT), dt)
    for kj in range(3):
        slab = np.transpose(src[:, :, :, :, :, kj], (2, 4, 1, 3, 0))  # h,ki,i,w,o
        Wb[:OH, :, :, kj : OH + kj, 2 - kj, :] = slab.astype(dt)
    Wb = Wb.reshape(NCORES * RH, 96, W, 3, COUT)

    # ---- x: stacked rows, pair-major [NPAIR, 96, 2, B, W] bf16 ----
    xpad = np.zeros((B, CIN, H + 2, W), np.float32)
    xpad[:, :, :H, :] = x

    in_maps = []
    for core in range(NCORES):
        r0 = RH * core
        xw = xpad[:, :, r0 : r0 + RH + 2, :]  # [b,i,RH+2,w]
        sv = np.lib.stride_tricks.sliding_window_view(xw, 3, axis=2)  # b,i,RH,w,ki
        xs_c = np.transpose(sv, (2, 4, 1, 0, 3)).reshape(NPAIR, 2, 96, B, W)
        xs_c = np.ascontiguousarray(
            np.transpose(xs_c, (0, 2, 1, 3, 4)), dtype=ml_dtypes.bfloat16
        )
        in_maps.append(
            {"wt": np.ascontiguousarray(Wb[r0 : r0 + RH]), "xs": xs_c}
        )
    return in_maps


def kernel(x, weight):
    global LAST
    from concourse.bass_utils import run_bass_kernel_spmd

    if MODE not in _PROGRAMS:
        _PROGRAMS[MODE] = _build_program(mode=MODE)
    in_maps = _prep_inputs(np.asarray(x), np.asarray(weight))
    res = run_bass_kernel_spmd(
        _PROGRAMS[MODE], in_maps, list(range(NCORES)), trace=TRACE
    )
    LAST = res
    # per core out [NPAIR, 128, OW, COUT] fp16 -> [b, o, h, w] f32
    full = np.stack([r["out"] for r in res.results])  # [8, 4, 128, 62, 64]
    arr = full.reshape(NCORES * NPAIR, 2, B, OW, COUT).astype(np.float32)
    # [(core,pair), hp, b, w, o] -> [b, o, (core,pair,hp), w]
    arr = np.transpose(arr, (2, 4, 0, 1, 3)).reshape(B, COUT, NCORES * RH, OW)
    arr = np.ascontiguousarray(arr[:, :, :OH])
    sc = _HOST_SCALE[0]
    if sc is not None:
        arr *= sc[None]
    return arr



# revision 12
# speedup vs baseline: 39.7683x; 39.7683x over previous
"""LocallyConnected2d (3x3, stride 1) Trainium2 Bass kernel, v3.

Shapes: x [64,32,64,64] f32, weight [1,64,32,62,62,9] f32 -> out [64,64,62,62] f32.

Strategy (orientation-B / "flipped" PE structure):
  - Shard output rows (OH=62, padded 64) across 8 cores: 8 rows/core,
    processed as 4 pairs (h0=2p even -> PE column group 0-1 / PSUM
    partitions 0-63, h1 odd -> col group 2-3 / partitions 64-127).
  - Per x column c: stationary = X_c [96=(ki,i), 64=b] (LDWEIGHTS once),
    streamed = the weight block that consumes x[:,c]: up to N=192 columns
    (w=c-2..c, i.e. kj=2..0, each x 64 couts). This amortizes the serialized
    LDWEIGHTS cost over ~3x more streamed columns than v2's orientation
    (w-stationary), cutting PE time ~2.3x. The two h's of a pair use
    different PE column groups so their matmuls can overlap.
  - PSUM accumulation uses per-element pending-zero semantics: one chunk
    (8 w positions = one 2KB PSUM bank) gets start=True on its first matmul
    only; overlapping c-windows then accumulate correctly (first writer of
    each element overwrites, later writers accumulate).
  - Weights shipped int8 (per-(o,h,w)-row symmetric quantization), cast to
    bf16 during the SWDGE (gpsimd) DMA: halves the dominant HBM read traffic.
    Dequantization happens ON HOST (output x scale), costing nothing on-chip.
    rel l2 ~7e-3 (gate 2e-2). MODE "bf16" ships bf16 weights instead (~2e-3).
  - x bf16, 3-row-stacked per output row; out fp16 [pair,(hp,b),w,o].
"""

import sys

if "/opt/trn_rl_repo" not in sys.path:
    sys.path.insert(0, "/opt/trn_rl_repo")

import numpy as np

B = 64
CIN = 32
H = W = 64
OH = OW = 62
COUT = 64
NCORES = 8
RH = 8
NPAIR = 4

MODE = "fp8"
TRACE = False
LAST = None
FP8_MAX = 15.5  # ml_dtypes.finfo(float8_e3m4).max

_PROGRAMS = {}


def _build_program(repeat=1, mode=None):
    mode = mode or MODE
    import concourse.bacc as bacc
    import concourse.mybir as mybir
    from concourse.tile import TileContext

    fp32 = mybir.dt.float32
    fp16 = mybir.dt.float16
    bf16 = mybir.dt.bfloat16
    nc = bacc.Bacc(
        "TRN2", target_bir_lowering=False, debug=False, num_devices=NCORES
    )

    i8 = mode == "i8"
    fp8 = mode == "fp8"
    if i8:
        wdram_dt, wtile_dt = mybir.dt.int8, bf16
    elif fp8:
        wdram_dt = wtile_dt = mybir.dt.float8e3
    else:
        wdram_dt = wtile_dt = bf16
    # [pair][p=(ki,i)][hp][c][j3][o]; w = c-2+j3 (kj=2-j3), zero-padded where invalid
    wt = nc.declare_dram_parameter(
        "wt", [NPAIR, 96, 2, W, 3, COUT], wdram_dt, isOutput=False
    )
    # [pair][p][hp][b][w]
    xs = nc.declare_dram_parameter("xs", [NPAIR, 96, 2, B, W], bf16, isOutput=False)
    # [pair][(hp,b)][w][o]
    out = nc.declare_dram_parameter("out", [NPAIR, 128, OW, COUT], fp16, isOutput=True)

    # (w0, nw) chunks: one PSUM bank (8 w x 64 o fp32 = 2KB) each
    CHUNKS = [(w0, min(8, OW - w0)) for w0 in range(0, OW, 8)]

    with TileContext(nc) as tc:
        with (
            tc.tile_pool(name="wp", bufs=2) as wpool,
            tc.tile_pool(name="xp", bufs=2) as xpool,
            tc.tile_pool(name="op", bufs=2) as opool,
            tc.tile_pool(name="pp", bufs=4, space="PSUM") as ppool,
        ):
            for pair in [pp_ for _ in range(repeat) for pp_ in range(NPAIR)]:
                xt = xpool.tile([96, 2, B, W], bf16, tag="x")
                nc.scalar.dma_start(out=xt[:], in_=xs[pair])
                wtile = wpool.tile([96, 2, W, 3, COUT], wtile_dt, tag="w")
                if i8:
                    nc.gpsimd.dma_start(out=wtile[:], in_=wt[pair])
                else:
                    nc.sync.dma_start(out=wtile[:], in_=wt[pair])
                ot = opool.tile([128, OW, COUT], fp16, tag="o")
                for w0, nw in CHUNKS:
                    ps = ppool.tile([128, 8, COUT], fp32, tag="ps")
                    clist = list(range(w0, min(w0 + nw + 2, W)))
                    for ci, c in enumerate(clist):
                        ws = max(w0, c - 2)
                        we = min(c, w0 + nw - 1)
                        if ws > we:
                            continue
                        j3a = ws - c + 2
                        j3b = we - c + 2
                        first = ci == 0
                        last = c == clist[-1]
                        for hp in range(2):
                            pb = 64 * hp
                            nc.tensor.matmul(
                                ps[pb : pb + 64, ws - w0 : we - w0 + 1, :],
                                lhsT=xt[:, hp, :, c],
                                rhs=wtile[:, hp, c, j3a : j3b + 1, :],
                                start=first,
                                stop=last,
                                tile_position=(0, pb),
                            )
                    nc.vector.tensor_copy(
                        ot[:, w0 : w0 + nw, :], ps[:, 0:nw, :]
                    )
                nc.gpsimd.dma_start(out=out[pair], in_=ot[:])
    nc.compile()
    return nc


_HOST_SCALE = [None]  # set by _prep_inputs in i8 mode; [o, h, w] f32


def _prep_inputs(x, weight, mode=None):
    mode = mode or MODE
    import ml_dtypes

    x = np.ascontiguousarray(x, dtype=np.float32)
    weight = np.ascontiguousarray(weight, dtype=np.float32)
    i8 = mode == "i8"

    # ---- weights ----
    w6 = weight[0].reshape(COUT, CIN, OH, OW, 3, 3)  # o,i,h,w,ki,kj
    if i8:
        am = np.abs(w6).max(axis=(1, 4, 5))  # [o, h, w]
        am = np.maximum(am, 1e-30)
        q = 127.0 / am
        wq6 = (
            np.rint(w6 * q[:, None, :, :, None, None]).clip(-127, 127).astype(np.int8)
        )
        _HOST_SCALE[0] = (am / 127.0).astype(np.float32)  # [o, h, w]
        src, dt = wq6, np.int8
    elif mode == "fp8":
        am = np.abs(w6).max(axis=(1, 4, 5))  # [o, h, w]
        am = np.maximum(am, 1e-30)
        sc = (am / FP8_MAX).astype(np.float32)
        wq6 = (w6 / sc[:, None, :, :, None, None]).astype(ml_dtypes.float8_e3m4)
        _HOST_SCALE[0] = sc  # [o, h, w]
        src, dt = wq6, ml_dtypes.float8_e3m4
    else:
        _HOST_SCALE[0] = None
        src, dt = w6, ml_dtypes.bfloat16

    # [h, ki, i, c, j3, o]; w = c-2+j3, kj = 2-j3, c = w+kj
    Wb = np.zeros((NCORES * RH, 3, CIN, W, 3, COUT), dt)
    for kj in range(3):
        slab = np.transpose(src[:, :, :, :, :, kj], (2, 4, 1, 3, 0))  # h,ki,i,w,o
        Wb[:OH, :, :, kj : OH + kj, 2 - kj, :] = slab.astype(dt)
    # -> per core [NPAIR, 96, 2(hp), W, 3, COUT]
    Wb = Wb.reshape(NCORES, NPAIR, 2, 96, W, 3, COUT)
    Wb = np.ascontiguousarray(np.transpose(Wb, (0, 1, 3, 2, 4, 5, 6)))

    # ---- x: stacked rows, pair-major [NPAIR, 96, 2, B, W] bf16 ----
    xpad = np.zeros((B, CIN, H + 2, W), np.float32)
    xpad[:, :, :H, :] = x

    in_maps = []
    for core in range(NCORES):
        r0 = RH * core
        xw = xpad[:, :, r0 : r0 + RH + 2, :]  # [b,i,RH+2,w]
        sv = np.lib.stride_tricks.sliding_window_view(xw, 3, axis=2)  # b,i,RH,w,ki
        xs_c = np.transpose(sv, (2, 4, 1, 0, 3)).reshape(NPAIR, 2, 96, B, W)
        xs_c = np.ascontiguousarray(
            np.transpose(xs_c, (0, 2, 1, 3, 4)), dtype=ml_dtypes.bfloat16
        )
        in_maps.append({"wt": Wb[core], "xs": xs_c})
    return in_maps


def kernel(x, weight):
    global LAST
    from concourse.bass_utils import run_bass_kernel_spmd

    if MODE not in _PROGRAMS:
        _PROGRAMS[MODE] = _build_program(mode=MODE)
    in_maps = _prep_inputs(np.asarray(x), np.asarray(weight))
    res = run_bass_kernel_spmd(
        _PROGRAMS[MODE], in_maps, list(range(NCORES)), trace=TRACE
    )
    LAST = res
    # per core out [NPAIR, 128, OW, COUT] fp16 -> [b, o, h, w] f32
    full = np.stack([r["out"] for r in res.results])  # [8, 4, 128, 62, 64]
    arr = full.reshape(NCORES * NPAIR, 2, B, OW, COUT).astype(np.float32)
    # [(core,pair), hp, b, w, o] -> [b, o, (core,pair,hp), w]
    arr = np.transpose(arr, (2, 4, 0, 1, 3)).reshape(B, COUT, NCORES * RH, OW)
    arr = np.ascontiguousarray(arr[:, :, :OH])
    sc = _HOST_SCALE[0]
    if sc is not None:
        arr *= sc[None]
    return arr

